# revision 83
# baseline (speedup 1.0000x reference)
"""GNN message-passing kernel for TRN2, one batch element per NeuronCore.

bf16 kernel, v3:
  - inputs bf16; weights packed into one DMA; x loaded in 4 w-slices on the
    SP queue while weights ride the ACT queue; gcn last
  - stats during load: xsum via tensor_scalar accum_out (4x), xmax via
    per-slice fold tree + incremental running max
  - S phase: ACT does sigmoid only; DVE does rowsums (accum_out), G1 copies,
    d-scale copies, Newton chains; colsums via PE ones-matmuls accumulate
    into deg_parts slot 2
  - d = rsqrt(deg): first Newton iter folded to one tensor_scalar from
    y0=1/32, one more stt-fused iter (5 ops per chain)
  - out_i = d_i * (xx_i @ T2) + G1_i: pairs alternate D-route (PE transpose
    of Pn + PE ident-add + ACT copy) and B-route (DVE scalar_tensor_tensor)
"""

from contextlib import ExitStack

import numpy as np

import concourse.bass as bass
import concourse.tile as tile
from concourse import bacc, mybir

f32 = mybir.dt.float32
bf16 = mybir.dt.bfloat16
AF = mybir.ActivationFunctionType
ALU = mybir.AluOpType

W, C, M = 2048, 512, 128
CQ = C // 128      # 4 c-chunks
NW = W // 128      # 16 w-chunks
WS = W // 512      # 4 w-slices

# ---- schedule tunables ----
G1_SCHED = {i: [i] for i in range(4)}
_nxt = 4
for _i in range(4, 10):
    G1_SCHED[_i] = [_nxt, _nxt + 1]
    _nxt += 2
OUT_ROUTE = {p: ("B" if p % 2 == 0 else "D") for p in range(8)}  # per pair
G1_COPY_ENG = {t: (1 if t in (0, 1, 2) else 0) for t in range(NW)}
ROWSUM_ENG = {i: (0 if i < 1 else 1) for i in range(NW)}


def build_nc():
    nc = bacc.Bacc("TRN2", target_bir_lowering=False, debug=False, num_devices=8)

    xT_d = nc.dram_tensor("xT", [C, W], bf16, kind="ExternalInput").ap()
    w3_d = nc.dram_tensor("w3", [C, 3 * M], bf16, kind="ExternalInput").ap()
    fcb_d = nc.dram_tensor("fcb", [M, 1], f32, kind="ExternalInput").ap()
    gcn_d = nc.dram_tensor("gcn", [C, C], bf16, kind="ExternalInput").ap()
    ident_d = nc.dram_tensor("ident", [128, 128], bf16, kind="ExternalInput").ap()
    out_d = nc.dram_tensor("out", [W, C], bf16, kind="ExternalOutput").ap()

    with tile.TileContext(nc) as tc, ExitStack() as ctx:
        pool = ctx.enter_context(tc.tile_pool(name="sb", bufs=1))
        sigp = ctx.enter_context(tc.tile_pool(name="sigp", bufs=4))
        outp = ctx.enter_context(tc.tile_pool(name="outp", bufs=8))
        tmpp = ctx.enter_context(tc.tile_pool(name="tmpp", bufs=4))
        psS = ctx.enter_context(tc.tile_pool(name="psS", bufs=2, space="PSUM"))
        psA = ctx.enter_context(tc.tile_pool(name="psA", bufs=2, space="PSUM"))
        psB = ctx.enter_context(tc.tile_pool(name="psB", bufs=1, space="PSUM"))
        psC = ctx.enter_context(tc.tile_pool(name="psC", bufs=1, space="PSUM"))

        # ---------- persistent SBUF tensors ----------
        xT = pool.tile([128, CQ, W], bf16)          # x^T, c-chunk k on partitions
        w3 = pool.tile([128, CQ, 3 * M], bf16)      # fcwT | avgwT | maxwT
        fcb = pool.tile([128, 1], f32)
        gcn = pool.tile([128, CQ, C], bf16)
        ident = pool.tile([128, 128], bf16)
        xxT = pool.tile([128, W], bf16)             # fc_w @ x^T + b   [M, W]
        dqT = pool.tile([128, W], bf16)             # cw * xxT
        Pn = pool.tile([128, NW, 128], bf16)        # d * xx, natural layout blocks
        G1 = pool.tile([128, NW, C], bf16)          # x @ gcn_w, w-chunk on partitions
        T2 = pool.tile([128, C], bf16)
        dumpD = pool.tile([128, CQ, 512], bf16)     # xsum dump
        dumpA = pool.tile([128, 2, 512], bf16)      # ACT xsum dump (slice 3)
        dumpR = pool.tile([128, W], bf16)           # rowsum dump
        SGACC = pool.tile([128, W], bf16)           # elementwise column accumulator
        xsum_p = pool.tile([128, CQ, WS], f32)
        foldA = pool.tile([128, CQ, 256], bf16)
        foldB = pool.tile([128, CQ, 128], bf16)
        foldC = pool.tile([128, CQ, 64], bf16)
        rmax = pool.tile([128, CQ, 64], bf16)       # running max across slices
        xsum_f = pool.tile([128, CQ], f32)
        xmax_f = pool.tile([128, CQ], f32)
        xsum16 = pool.tile([128, CQ], bf16)
        xmax16 = pool.tile([128, CQ], bf16)
        a_sb = pool.tile([128, 1], f32)
        m_sb = pool.tile([128, 1], f32)
        cw = pool.tile([128, 1], f32)
        ncw = pool.tile([128, 1], f32)
        ones16 = pool.tile([128, 1], bf16)
        zeros1 = pool.tile([128, 1], f32)
        scr1 = pool.tile([128, 1], f32)
        deg_parts = pool.tile([128, NW, 2], f32)    # rowsumA | rowsumB
        deg = pool.tile([128, NW], f32)
        y_nr = pool.tile([128, NW], f32)            # rsqrt iterate -> d
        t_nr = pool.tile([128, NW], f32)
        u_nr = pool.tile([128, NW], f32)

        fcwT = w3[:, :, 0:M]
        avgwT = w3[:, :, M : 2 * M]
        maxwT = w3[:, :, 2 * M : 3 * M]

        # Pin the ACT table set: make the first ACT instruction a Sigmoid.
        nc.gpsimd.memset(zeros1[:], 0.0)
        nc.scalar.activation(scr1[:], zeros1[:], AF.Sigmoid)
        nc.gpsimd.memset(SGACC[:], 0.0)
        nc.vector.memset(ones16[:], 1.0)
        nc.vector.memset(deg_parts[:].rearrange("p a b -> p (a b)"), 0.0)

        # PE warmup: ramp the PE pstate with wide matmuls on memset data
        for _wu in range(10):
            pw = psS.tile([128, 1024], f32, tag="s")
            nc.tensor.matmul(pw[:, 0:512], SGACC[:, 0:128], SGACC[:, 0:512],
                             start=True, stop=True)

        # ---------- loads: x on SP queue; weights on ACT queue; gcn last ----------
        nc.scalar.dma_start(fcb[:], fcb_d[:])
        nc.scalar.dma_start(ident[:], ident_d[:])
        nc.scalar.dma_start(w3[:], w3_d.rearrange("(k p) m -> p k m", p=128))
        for s in range(WS):
            nc.sync.dma_start(
                xT[:, :, bass.ts(s, 512)],
                xT_d[:, bass.ts(s, 512)].rearrange("(k p) w -> p k w", p=128),
            )
        nc.sync.dma_start(gcn[:], gcn_d.rearrange("(k p) c -> p k c", p=128))

        # ---------- per-slice stats + xxT (overlap the x load) ----------
        def slice_stats(s):
            sl = xT[:, :, bass.ts(s, 512)]
            for k in range(CQ):
                nc.vector.tensor_scalar(
                    dumpD[:, k, :], xT[:, k, bass.ts(s, 512)], 1.0, 0.0,
                    op0=ALU.mult, op1=ALU.add, accum_out=xsum_p[:, k, s : s + 1],
                )
            v = sl.rearrange("p k (h w) -> p k h w", h=2)
            nc.vector.tensor_tensor(foldA[:], v[:, :, 0, :], v[:, :, 1, :], op=ALU.max)
            vA = foldA.rearrange("p k (h w) -> p k h w", h=2)
            nc.vector.tensor_tensor(foldB[:], vA[:, :, 0, :], vA[:, :, 1, :], op=ALU.max)
            vB = foldB.rearrange("p k (h w) -> p k h w", h=2)
            nc.vector.tensor_tensor(foldC[:], vB[:, :, 0, :], vB[:, :, 1, :], op=ALU.max)
            if s == 0:
                nc.vector.tensor_copy(rmax[:], foldC[:])
            else:
                nc.vector.tensor_tensor(rmax[:], rmax[:], foldC[:], op=ALU.max)

        def xxt_slice(s):
            px = psA.tile([128, 512], f32, tag="a")
            for k in range(CQ):
                nc.tensor.matmul(
                    px[:], fcwT[:, k, :], xT[:, k, bass.ts(s, 512)],
                    start=(k == 0), stop=(k == CQ - 1),
                )
            nc.scalar.activation(xxT[:, bass.ts(s, 512)], px[:], AF.Identity, bias=fcb[:, 0:1])

        for s in range(WS):
            slice_stats(s)
            xxt_slice(s)

        # ---------- cw ----------
        nc.vector.reduce_sum(xsum_f[:], xsum_p[:], axis=mybir.AxisListType.X)
        nc.vector.tensor_copy(xsum16[:], xsum_f[:])
        nc.vector.reduce_max(xmax_f[:], rmax[:], axis=mybir.AxisListType.X)
        nc.vector.tensor_copy(xmax16[:], xmax_f[:])
        pa = psB.tile([128, 512], f32, tag="b")
        for k in range(CQ):
            nc.tensor.matmul(pa[:, 0:1], avgwT[:, k, :], xsum16[:, k : k + 1],
                             start=(k == 0), stop=(k == CQ - 1))
        nc.scalar.activation(a_sb[:], pa[:, 0:1], AF.Relu, scale=1.0 / W)
        pm = psB.tile([128, 512], f32, tag="b")
        for k in range(CQ):
            nc.tensor.matmul(pm[:, 0:1], maxwT[:, k, :], xmax16[:, k : k + 1],
                             start=(k == 0), stop=(k == CQ - 1))
        nc.scalar.activation(m_sb[:], pm[:, 0:1], AF.Relu)
        nc.scalar.activation(cw[:], a_sb[:], AF.Sigmoid, bias=m_sb[:, 0:1])
        nc.vector.tensor_scalar_mul(ncw[:], cw[:], -1.0)

        # dqT = cw * xxT (block 0 first so S starts immediately); 4x DVE
        nc.vector.tensor_scalar_mul(dqT[:, 0:128], xxT[:, 0:128], cw[:, 0:1])
        nc.vector.tensor_scalar_mul(dqT[:, 128:1024], xxT[:, 128:1024], cw[:, 0:1])
        nc.vector.tensor_scalar_mul(dqT[:, 1024:2048], xxT[:, 1024:2048], cw[:, 0:1])

        pt1 = None

        def g1_tile(t):
            pg = psA.tile([128, 512], f32, tag="a")
            for k in range(CQ):
                nc.tensor.matmul(pg[:], xT[:, k, bass.ts(t, 128)], gcn[:, k, :],
                                 start=(k == 0), stop=(k == CQ - 1))
            if G1_COPY_ENG[t] == 0:
                nc.vector.tensor_copy(G1[:, t, :], pg[:])
            else:
                nc.scalar.activation(G1[:, t, :], pg[:], AF.Copy)

        psCt = None

        def colsum_chunks(lo, hi):
            """Column sums for chunks [lo,hi) from SGACC via one matmul each."""
            nonlocal psCt
            if psCt is None:
                psCt = psC.tile([128, NW], f32, tag="c")
            for j in range(max(lo, 1), hi):
                nc.tensor.matmul(psCt[:, j : j + 1], SGACC[:, bass.ts(j, 128)], ones16[:],
                                 start=True, stop=True)

        def d_chain(lo, hi):
            """d[lo:hi] = rsqrt(deg) via y0=1/32 + 2 fused Newton iters."""
            sl = slice(lo, hi)
            nc.vector.reduce_sum(deg[:, sl], deg_parts[:, sl, :], axis=mybir.AxisListType.X)
            csl = slice(max(lo, 1), hi)
            nc.vector.tensor_tensor(deg[:, csl], deg[:, csl], psCt[:, csl], op=ALU.add)
            # y1 = y0*(1.5 - 0.5*deg*y0^2), y0 = 1/32
            nc.vector.tensor_scalar(
                y_nr[:, sl], deg[:, sl], -1.0 / 65536.0, 3.0 / 64.0, op0=ALU.mult, op1=ALU.add
            )
            nc.vector.tensor_tensor(t_nr[:, sl], y_nr[:, sl], y_nr[:, sl], op=ALU.mult)
            nc.vector.scalar_tensor_tensor(
                u_nr[:, sl], t_nr[:, sl], -0.5, deg[:, sl], op0=ALU.mult, op1=ALU.mult
            )
            nc.vector.scalar_tensor_tensor(
                y_nr[:, sl], u_nr[:, sl], 1.5, y_nr[:, sl], op0=ALU.add, op1=ALU.mult
            )

        def p_group(lo, hi):
            """PE-transpose xxT blocks (bf16 psum) + fused d-scale copy -> Pn."""
            tp = psA.tile([128, 512], f32, tag="a")
            for i in range(lo, hi):
                q = i - lo
                nc.tensor.transpose(
                    tp[:, 64 * q : 64 * (q + 1)].bitcast(bf16), xxT[:, bass.ts(i, 128)], ident[:]
                )
            for i in range(lo, hi):
                q = i - lo
                nc.vector.tensor_scalar_mul(
                    Pn[:, i, :], tp[:, 64 * q : 64 * (q + 1)].bitcast(bf16), y_nr[:, i : i + 1]
                )

        def t1_mms(lo, hi):
            nonlocal pt1
            if pt1 is None:
                pt1 = psB.tile([128, 512], f32, tag="b")
            for i in range(lo, hi):
                nc.tensor.matmul(pt1[:], Pn[:, i, :], G1[:, i, :], start=(i == 0), stop=(i == NW - 1))

        # ---------- S phase ----------
        for i in range(NW):
            start_col = 128 * i
            width = W - start_col
            parts = []
            c0 = start_col
            if width > 1024:
                parts.append((c0, width - 1024))
                parts.append((c0 + width - 1024, 1024))
            else:
                parts.append((c0, width))
            sig_tiles = []
            for pidx, (c0, w) in enumerate(parts):
                ps = psS.tile([128, 1024], f32, tag="s")
                o = 0
                while o < w:
                    n = min(512, w - o)
                    nc.tensor.matmul(
                        ps[:, o : o + n], dqT[:, bass.ts(i, 128)],
                        xxT[:, c0 + o : c0 + o + n], start=True, stop=True,
                    )
                    o += n
                sg = sigp.tile([128, 1024], bf16, tag="sg")
                if ROWSUM_ENG[i] == 1:
                    nc.scalar.activation(
                        sg[:, 0:w], ps[:, 0:w], AF.Sigmoid,
                        accum_out=deg_parts[:, i, pidx : pidx + 1],
                    )
                else:
                    nc.scalar.activation(sg[:, 0:w], ps[:, 0:w], AF.Sigmoid)
                    nc.vector.tensor_scalar(
                        dumpR[:, c0 : c0 + w], sg[:, 0:w], 1.0, 0.0,
                        op0=ALU.mult, op1=ALU.add, accum_out=deg_parts[:, i, pidx : pidx + 1],
                    )
                sig_tiles.append((sg, c0, w))
            # off-diagonal columns: elementwise accumulate into SGACC
            if i < NW - 1:
                off0 = 128 * (i + 1)
                for sg, c0, w in sig_tiles:
                    a0 = max(c0, off0)
                    if a0 < c0 + w:
                        nc.vector.tensor_tensor(
                            SGACC[:, a0 : c0 + w], SGACC[:, a0 : c0 + w],
                            sg[:, a0 - c0 : w], op=ALU.add,
                        )
            for gi in G1_SCHED.get(i, []):
                g1_tile(gi)
            if i == 8:
                colsum_chunks(0, 8)
                d_chain(0, 8)
            if i == 12:
                colsum_chunks(8, 12)
                d_chain(8, 12)
            if i == 9:
                p_group(0, 8)
                t1_mms(0, 8)
            if i == 13:
                p_group(8, 12)
                t1_mms(8, 12)
            if i == 15:
                colsum_chunks(12, 15)
                d_chain(12, 15)
                p_group(12, 15)
                t1_mms(12, 15)

        colsum_chunks(15, 16)
        d_chain(15, 16)
        p_group(15, 16)
        t1_mms(15, 16)

        # T2 = (-cw) * T1
        nc.vector.tensor_scalar_mul(T2[:], pt1[:], ncw[:, 0:1])

        # ---------- out pairs ----------
        for p in range(8):
            st = outp.tile([128, 2, 512], bf16)
            py2 = psS.tile([128, 1024], f32, tag="s")
            route = OUT_ROUTE[p]
            if route == "D":
                tp = psA.tile([128, 512], f32, tag="a")
                PTsb = tmpp.tile([128, 256], bf16, tag="t")
                for q in range(2):
                    i = 2 * p + q
                    nc.tensor.transpose(
                        tp[:, 64 * q : 64 * (q + 1)].bitcast(bf16), Pn[:, i, :], ident[:]
                    )
                for q in range(2):
                    nc.vector.tensor_copy(
                        PTsb[:, bass.ts(q, 128)], tp[:, 64 * q : 64 * (q + 1)].bitcast(bf16)
                    )
                for q in range(2):
                    i = 2 * p + q
                    nc.tensor.matmul(py2[:, bass.ts(q, 512)], PTsb[:, bass.ts(q, 128)], T2[:],
                                     start=True, stop=False)
                    nc.tensor.matmul(py2[:, bass.ts(q, 512)], ident[:], G1[:, i, :],
                                     start=False, stop=True)
                nc.scalar.activation(
                    st[:].rearrange("p a b -> p (a b)"), py2[:], AF.Copy
                )
            else:
                for q in range(2):
                    i = 2 * p + q
                    nc.tensor.matmul(py2[:, bass.ts(q, 512)], xxT[:, bass.ts(i, 128)], T2[:],
                                     start=True, stop=True)
                for q in range(2):
                    i = 2 * p + q
                    nc.vector.scalar_tensor_tensor(
                        st[:, q, :], py2[:, bass.ts(q, 512)], y_nr[:, i : i + 1], G1[:, i, :],
                        op0=ALU.mult, op1=ALU.add,
                    )
            eng = nc.sync if p % 2 == 0 else nc.gpsimd
            eng.dma_start(
                out_d[bass.ts(p, 256), :].rearrange("(q p) c -> p q c", p=128), st[:]
            )

    nc.compile()
    return nc


# ======================================================================
# Harness entry point: full inputs in, full output out.
# Shards batch B=8 across the 8 NeuronCores (pure data parallel).
# ======================================================================

_NC_CACHE = None


def _get_nc():
    global _NC_CACHE
    if _NC_CACHE is None:
        _NC_CACHE = build_nc()
    return _NC_CACHE


def make_in_maps(x, fc_w, fc_b, avg_w, max_w, gcn_w):
    import ml_dtypes

    b16 = ml_dtypes.bfloat16
    x = np.asarray(x, dtype=np.float32)
    fc_w = np.asarray(fc_w, dtype=np.float32)
    fc_b = np.asarray(fc_b, dtype=np.float32)
    avg_w = np.asarray(avg_w, dtype=np.float32)
    max_w = np.asarray(max_w, dtype=np.float32)
    gcn_w = np.asarray(gcn_w, dtype=np.float32)
    w3 = np.concatenate(
        [np.ascontiguousarray(fc_w.T), np.ascontiguousarray(avg_w.T),
         np.ascontiguousarray(max_w.T)], axis=1,
    )
    shared = {
        "w3": np.ascontiguousarray(w3).astype(b16),
        "fcb": np.ascontiguousarray(fc_b.reshape(M, 1)),
        "gcn": np.ascontiguousarray(gcn_w).astype(b16),
        "ident": np.eye(128, dtype=np.float32).astype(b16),
    }
    return [
        {"xT": np.ascontiguousarray(x[b].T).astype(b16), **shared}
        for b in range(x.shape[0])
    ]


def kernel(x, fc_w, fc_b, avg_w, max_w, gcn_w):
    from concourse.bass_utils import run_bass_kernel_spmd

    nc = _get_nc()
    in_maps = make_in_maps(x, fc_w, fc_b, avg_w, max_w, gcn_w)
    res = run_bass_kernel_spmd(nc, in_maps, list(range(len(in_maps))))
    out = np.stack([np.asarray(res.results[b]["out"]) for b in range(len(in_maps))])
    return out.astype(np.float32)


# revision 87
# speedup vs baseline: 1.0010x; 1.0010x over previous
"""GNN message-passing kernel for TRN2, one batch element per NeuronCore.

bf16 kernel, v3:
  - inputs bf16; weights packed into one DMA; x loaded in 4 w-slices on the
    SP queue while weights ride the ACT queue; gcn last
  - stats during load: xsum via tensor_scalar accum_out (4x), xmax via
    per-slice fold tree + incremental running max
  - S phase: ACT does sigmoid only; DVE does rowsums (accum_out), G1 copies,
    d-scale copies, Newton chains; colsums via PE ones-matmuls accumulate
    into deg_parts slot 2
  - d = rsqrt(deg): first Newton iter folded to one tensor_scalar from
    y0=1/32, one more stt-fused iter (5 ops per chain)
  - out_i = d_i * (xx_i @ T2) + G1_i: pairs alternate D-route (PE transpose
    of Pn + PE ident-add + ACT copy) and B-route (DVE scalar_tensor_tensor)
"""

from contextlib import ExitStack

import numpy as np

import concourse.bass as bass
import concourse.tile as tile
from concourse import bacc, mybir

f32 = mybir.dt.float32
bf16 = mybir.dt.bfloat16
AF = mybir.ActivationFunctionType
ALU = mybir.AluOpType

W, C, M = 2048, 512, 128
CQ = C // 128      # 4 c-chunks
NW = W // 128      # 16 w-chunks
WS = W // 512      # 4 w-slices

# ---- schedule tunables ----
G1_SCHED = {i: [i] for i in range(4)}
_nxt = 4
for _i in range(4, 10):
    G1_SCHED[_i] = [_nxt, _nxt + 1]
    _nxt += 2
OUT_ROUTE = {p: ("B" if p % 2 == 0 else "D") for p in range(8)}  # per pair
G1_COPY_ENG = {t: (1 if t in (0, 1, 2) else 0) for t in range(NW)}
ROWSUM_ENG = {i: (0 if i < 1 else 1) for i in range(NW)}


def build_nc():
    nc = bacc.Bacc("TRN2", target_bir_lowering=False, debug=False, num_devices=8)

    xT_d = nc.dram_tensor("xT", [C, W], bf16, kind="ExternalInput").ap()
    w3_d = nc.dram_tensor("w3", [C, 3 * M], bf16, kind="ExternalInput").ap()
    fcb_d = nc.dram_tensor("fcb", [M, 1], f32, kind="ExternalInput").ap()
    gcn_d = nc.dram_tensor("gcn", [C, C], bf16, kind="ExternalInput").ap()
    ident_d = nc.dram_tensor("ident", [128, 128], bf16, kind="ExternalInput").ap()
    out_d = nc.dram_tensor("out", [W, C], bf16, kind="ExternalOutput").ap()

    with tile.TileContext(nc) as tc, ExitStack() as ctx:
        pool = ctx.enter_context(tc.tile_pool(name="sb", bufs=1))
        sigp = ctx.enter_context(tc.tile_pool(name="sigp", bufs=4))
        outp = ctx.enter_context(tc.tile_pool(name="outp", bufs=8))
        tmpp = ctx.enter_context(tc.tile_pool(name="tmpp", bufs=4))
        psS = ctx.enter_context(tc.tile_pool(name="psS", bufs=2, space="PSUM"))
        psA = ctx.enter_context(tc.tile_pool(name="psA", bufs=2, space="PSUM"))
        psB = ctx.enter_context(tc.tile_pool(name="psB", bufs=1, space="PSUM"))
        psC = ctx.enter_context(tc.tile_pool(name="psC", bufs=1, space="PSUM"))

        # ---------- persistent SBUF tensors ----------
        xT = pool.tile([128, CQ, W], bf16)          # x^T, c-chunk k on partitions
        w3 = pool.tile([128, CQ, 3 * M], bf16)      # fcwT | avgwT | maxwT
        fcb = pool.tile([128, 1], f32)
        gcn = pool.tile([128, CQ, C], bf16)
        ident = pool.tile([128, 128], bf16)
        xxT = pool.tile([128, W], bf16)             # fc_w @ x^T + b   [M, W]
        dqT = pool.tile([128, W], bf16)             # cw * xxT
        Pn = pool.tile([128, NW, 128], bf16)        # d * xx, natural layout blocks
        G1 = pool.tile([128, NW, C], bf16)          # x @ gcn_w, w-chunk on partitions
        T2 = pool.tile([128, C], bf16)
        dumpD = pool.tile([128, CQ, 512], bf16)     # xsum dump
        dumpA = pool.tile([128, 2, 512], bf16)      # ACT xsum dump (slice 3)
        dumpR = pool.tile([128, W], bf16)           # rowsum dump
        SGACC = pool.tile([128, W], bf16)           # elementwise column accumulator
        xsum_p = pool.tile([128, CQ, WS], f32)
        foldA = pool.tile([128, CQ, 256], bf16)
        foldB = pool.tile([128, CQ, 128], bf16)
        foldC = pool.tile([128, CQ, 64], bf16)
        rmax = pool.tile([128, CQ, 64], bf16)       # running max across slices
        xsum_f = pool.tile([128, CQ], f32)
        xmax_f = pool.tile([128, CQ], f32)
        xsum16 = pool.tile([128, CQ], bf16)
        xmax16 = pool.tile([128, CQ], bf16)
        a_sb = pool.tile([128, 1], f32)
        m_sb = pool.tile([128, 1], f32)
        cw = pool.tile([128, 1], f32)
        ncw = pool.tile([128, 1], f32)
        ones16 = pool.tile([128, 1], bf16)
        zeros1 = pool.tile([128, 1], f32)
        scr1 = pool.tile([128, 1], f32)
        deg_parts = pool.tile([128, NW, 2], f32)    # rowsumA | rowsumB
        deg = pool.tile([128, NW], f32)
        y_nr = pool.tile([128, NW], f32)            # rsqrt iterate -> d
        t_nr = pool.tile([128, NW], f32)
        u_nr = pool.tile([128, NW], f32)

        fcwT = w3[:, :, 0:M]
        avgwT = w3[:, :, M : 2 * M]
        maxwT = w3[:, :, 2 * M : 3 * M]

        # Pin the ACT table set: make the first ACT instruction a Sigmoid.
        nc.gpsimd.memset(zeros1[:], 0.0)
        nc.scalar.activation(scr1[:], zeros1[:], AF.Sigmoid)
        nc.gpsimd.memset(SGACC[:], 0.0)
        nc.vector.memset(ones16[:], 1.0)
        nc.vector.memset(deg_parts[:].rearrange("p a b -> p (a b)"), 0.0)

        # PE warmup: ramp the PE pstate with wide matmuls on memset data
        for _wu in range(10):
            pw = psS.tile([128, 1024], f32, tag="s")
            nc.tensor.matmul(pw[:, 0:512], SGACC[:, 0:128], SGACC[:, 0:512],
                             start=True, stop=True)

        # ---------- loads: x on SP queue; weights on ACT queue; gcn last ----------
        nc.scalar.dma_start(fcb[:], fcb_d[:])
        nc.scalar.dma_start(ident[:], ident_d[:])
        nc.scalar.dma_start(w3[:], w3_d.rearrange("(k p) m -> p k m", p=128))
        for s in range(WS):
            nc.sync.dma_start(
                xT[:, :, bass.ts(s, 512)],
                xT_d[:, bass.ts(s, 512)].rearrange("(k p) w -> p k w", p=128),
            )
        nc.sync.dma_start(gcn[:], gcn_d.rearrange("(k p) c -> p k c", p=128))

        # ---------- per-slice stats + xxT (overlap the x load) ----------
        def slice_stats(s):
            sl = xT[:, :, bass.ts(s, 512)]
            for k in range(CQ):
                nc.vector.tensor_scalar(
                    dumpD[:, k, :], xT[:, k, bass.ts(s, 512)], 1.0, 0.0,
                    op0=ALU.mult, op1=ALU.add, accum_out=xsum_p[:, k, s : s + 1],
                )
            v = sl.rearrange("p k (h w) -> p k h w", h=2)
            nc.vector.tensor_tensor(foldA[:], v[:, :, 0, :], v[:, :, 1, :], op=ALU.max)
            vA = foldA.rearrange("p k (h w) -> p k h w", h=2)
            nc.vector.tensor_tensor(foldB[:], vA[:, :, 0, :], vA[:, :, 1, :], op=ALU.max)
            vB = foldB.rearrange("p k (h w) -> p k h w", h=2)
            nc.vector.tensor_tensor(foldC[:], vB[:, :, 0, :], vB[:, :, 1, :], op=ALU.max)
            if s == 0:
                nc.vector.tensor_copy(rmax[:], foldC[:])
            else:
                nc.vector.tensor_tensor(rmax[:], rmax[:], foldC[:], op=ALU.max)

        def xxt_slice(s):
            px = psA.tile([128, 512], f32, tag="a")
            for k in range(CQ):
                nc.tensor.matmul(
                    px[:], fcwT[:, k, :], xT[:, k, bass.ts(s, 512)],
                    start=(k == 0), stop=(k == CQ - 1),
                )
            nc.scalar.activation(xxT[:, bass.ts(s, 512)], px[:], AF.Identity, bias=fcb[:, 0:1])

        for s in range(WS):
            slice_stats(s)
            xxt_slice(s)

        # ---------- cw ----------
        nc.vector.reduce_sum(xsum_f[:], xsum_p[:], axis=mybir.AxisListType.X)
        nc.vector.tensor_copy(xsum16[:], xsum_f[:])
        nc.vector.reduce_max(xmax_f[:], rmax[:], axis=mybir.AxisListType.X)
        nc.vector.tensor_copy(xmax16[:], xmax_f[:])
        pa = psB.tile([128, 512], f32, tag="b")
        for k in range(CQ):
            nc.tensor.matmul(pa[:, 0:1], avgwT[:, k, :], xsum16[:, k : k + 1],
                             start=(k == 0), stop=(k == CQ - 1))
        nc.scalar.activation(a_sb[:], pa[:, 0:1], AF.Relu, scale=1.0 / W)
        pm = psB.tile([128, 512], f32, tag="b")
        for k in range(CQ):
            nc.tensor.matmul(pm[:, 0:1], maxwT[:, k, :], xmax16[:, k : k + 1],
                             start=(k == 0), stop=(k == CQ - 1))
        nc.scalar.activation(m_sb[:], pm[:, 0:1], AF.Relu)
        nc.scalar.activation(cw[:], a_sb[:], AF.Sigmoid, bias=m_sb[:, 0:1])
        nc.vector.tensor_scalar_mul(ncw[:], cw[:], -1.0)

        # dqT = cw * xxT (block 0 first so S starts immediately); 4x DVE
        nc.vector.tensor_scalar_mul(dqT[:, 0:128], xxT[:, 0:128], cw[:, 0:1])
        nc.vector.tensor_scalar_mul(dqT[:, 128:1024], xxT[:, 128:1024], cw[:, 0:1])
        nc.vector.tensor_scalar_mul(dqT[:, 1024:2048], xxT[:, 1024:2048], cw[:, 0:1])

        pt1 = None

        def g1_tile(t):
            pg = psA.tile([128, 512], f32, tag="a")
            for k in range(CQ):
                nc.tensor.matmul(pg[:], xT[:, k, bass.ts(t, 128)], gcn[:, k, :],
                                 start=(k == 0), stop=(k == CQ - 1))
            if G1_COPY_ENG[t] == 0:
                nc.vector.tensor_copy(G1[:, t, :], pg[:])
            else:
                nc.scalar.activation(G1[:, t, :], pg[:], AF.Copy)

        psCt = None

        def colsum_chunks(lo, hi):
            """Column sums for chunks [lo,hi) from SGACC via one matmul each."""
            nonlocal psCt
            if psCt is None:
                psCt = psC.tile([128, NW], f32, tag="c")
            for j in range(max(lo, 1), hi):
                nc.tensor.matmul(psCt[:, j : j + 1], SGACC[:, bass.ts(j, 128)], ones16[:],
                                 start=True, stop=True)

        def d_chain(lo, hi):
            """d[lo:hi] = rsqrt(deg) via y0=1/32 + 2 fused Newton iters."""
            sl = slice(lo, hi)
            nc.vector.reduce_sum(deg[:, sl], deg_parts[:, sl, :], axis=mybir.AxisListType.X)
            csl = slice(max(lo, 1), hi)
            nc.vector.tensor_tensor(deg[:, csl], deg[:, csl], psCt[:, csl], op=ALU.add)
            # y1 = y0*(1.5 - 0.5*deg*y0^2), y0 = 1/32
            nc.vector.tensor_scalar(
                y_nr[:, sl], deg[:, sl], -1.0 / 65536.0, 3.0 / 64.0, op0=ALU.mult, op1=ALU.add
            )
            nc.vector.tensor_tensor(t_nr[:, sl], y_nr[:, sl], y_nr[:, sl], op=ALU.mult)
            nc.vector.scalar_tensor_tensor(
                u_nr[:, sl], t_nr[:, sl], -0.5, deg[:, sl], op0=ALU.mult, op1=ALU.mult
            )
            nc.vector.scalar_tensor_tensor(
                y_nr[:, sl], u_nr[:, sl], 1.5, y_nr[:, sl], op0=ALU.add, op1=ALU.mult
            )

        def p_group(lo, hi):
            """PE-transpose xxT blocks (bf16 psum) + fused d-scale copy -> Pn."""
            tp = psA.tile([128, 512], f32, tag="a")
            for i in range(lo, hi):
                q = i - lo
                nc.tensor.transpose(
                    tp[:, 64 * q : 64 * (q + 1)].bitcast(bf16), xxT[:, bass.ts(i, 128)], ident[:]
                )
            for i in range(lo, hi):
                q = i - lo
                nc.vector.tensor_scalar_mul(
                    Pn[:, i, :], tp[:, 64 * q : 64 * (q + 1)].bitcast(bf16), y_nr[:, i : i + 1]
                )

        def t1_mms(lo, hi):
            nonlocal pt1
            if pt1 is None:
                pt1 = psB.tile([128, 512], f32, tag="b")
            for i in range(lo, hi):
                nc.tensor.matmul(pt1[:], Pn[:, i, :], G1[:, i, :], start=(i == 0), stop=(i == NW - 1))

        # ---------- S phase ----------
        last_sg = {}
        for i in range(NW):
            start_col = 128 * i
            width = W - start_col
            parts = []
            c0 = start_col
            if width > 1024:
                parts.append((c0, width - 1024))
                parts.append((c0 + width - 1024, 1024))
            else:
                parts.append((c0, width))
            sig_tiles = []
            for pidx, (c0, w) in enumerate(parts):
                ps = psS.tile([128, 1024], f32, tag="s")
                o = 0
                while o < w:
                    n = min(512, w - o)
                    nc.tensor.matmul(
                        ps[:, o : o + n], dqT[:, bass.ts(i, 128)],
                        xxT[:, c0 + o : c0 + o + n], start=True, stop=True,
                    )
                    o += n
                sg = sigp.tile([128, 1024], bf16, tag="sg")
                if ROWSUM_ENG[i] == 1:
                    nc.scalar.activation(
                        sg[:, 0:w], ps[:, 0:w], AF.Sigmoid,
                        accum_out=deg_parts[:, i, pidx : pidx + 1],
                    )
                else:
                    nc.scalar.activation(sg[:, 0:w], ps[:, 0:w], AF.Sigmoid)
                    nc.vector.tensor_scalar(
                        dumpR[:, c0 : c0 + w], sg[:, 0:w], 1.0, 0.0,
                        op0=ALU.mult, op1=ALU.add, accum_out=deg_parts[:, i, pidx : pidx + 1],
                    )
                sig_tiles.append((sg, c0, w))
                last_sg[i] = (sg, c0, w)
            # off-diagonal columns: elementwise accumulate into SGACC
            if i < NW - 2:
                off0 = 128 * (i + 1)
                for sg, c0, w in sig_tiles:
                    a0 = max(c0, off0)
                    if a0 < c0 + w:
                        nc.vector.tensor_tensor(
                            SGACC[:, a0 : c0 + w], SGACC[:, a0 : c0 + w],
                            sg[:, a0 - c0 : w], op=ALU.add,
                        )
            for gi in G1_SCHED.get(i, []):
                g1_tile(gi)
            if i == 8:
                colsum_chunks(0, 8)
                d_chain(0, 8)
            if i == 12:
                colsum_chunks(8, 12)
                d_chain(8, 12)
            if i == 9:
                p_group(0, 8)
                t1_mms(0, 8)
            if i == 13:
                p_group(8, 12)
                t1_mms(8, 12)
            if i == 15:
                colsum_chunks(12, 15)
                d_chain(12, 15)
                p_group(12, 15)
                t1_mms(12, 15)

        nc.tensor.matmul(psCt[:, 15:16], SGACC[:, bass.ts(15, 128)], ones16[:],
                         start=True, stop=False)
        sg14, c14, w14 = last_sg[14]
        nc.tensor.matmul(psCt[:, 15:16], sg14[:, 1920 - c14 : 2048 - c14], ones16[:],
                         start=False, stop=True)
        d_chain(15, 16)
        p_group(15, 16)
        t1_mms(15, 16)

        # T2 = (-cw) * T1
        nc.vector.tensor_scalar_mul(T2[:], pt1[:], ncw[:, 0:1])

        # ---------- out pairs ----------
        for p in range(8):
            st = outp.tile([128, 2, 512], bf16)
            py2 = psS.tile([128, 1024], f32, tag="s")
            route = OUT_ROUTE[p]
            if route == "D":
                tp = psA.tile([128, 512], f32, tag="a")
                PTsb = tmpp.tile([128, 256], bf16, tag="t")
                for q in range(2):
                    i = 2 * p + q
                    nc.tensor.transpose(
                        tp[:, 64 * q : 64 * (q + 1)].bitcast(bf16), Pn[:, i, :], ident[:]
                    )
                for q in range(2):
                    nc.vector.tensor_copy(
                        PTsb[:, bass.ts(q, 128)], tp[:, 64 * q : 64 * (q + 1)].bitcast(bf16)
                    )
                for q in range(2):
                    i = 2 * p + q
                    nc.tensor.matmul(py2[:, bass.ts(q, 512)], PTsb[:, bass.ts(q, 128)], T2[:],
                                     start=True, stop=False)
                    nc.tensor.matmul(py2[:, bass.ts(q, 512)], ident[:], G1[:, i, :],
                                     start=False, stop=True)
                nc.scalar.activation(
                    st[:].rearrange("p a b -> p (a b)"), py2[:], AF.Copy
                )
            else:
                for q in range(2):
                    i = 2 * p + q
                    nc.tensor.matmul(py2[:, bass.ts(q, 512)], xxT[:, bass.ts(i, 128)], T2[:],
                                     start=True, stop=True)
                for q in range(2):
                    i = 2 * p + q
                    nc.vector.scalar_tensor_tensor(
                        st[:, q, :], py2[:, bass.ts(q, 512)], y_nr[:, i : i + 1], G1[:, i, :],
                        op0=ALU.mult, op1=ALU.add,
                    )
            eng = nc.sync if p % 2 == 0 else nc.gpsimd
            eng.dma_start(
                out_d[bass.ts(p, 256), :].rearrange("(q p) c -> p q c", p=128), st[:]
            )

    nc.compile()
    return nc


# ======================================================================
# Harness entry point: full inputs in, full output out.
# Shards batch B=8 across the 8 NeuronCores (pure data parallel).
# ======================================================================

_NC_CACHE = None


def _get_nc():
    global _NC_CACHE
    if _NC_CACHE is None:
        _NC_CACHE = build_nc()
    return _NC_CACHE


def make_in_maps(x, fc_w, fc_b, avg_w, max_w, gcn_w):
    import ml_dtypes

    b16 = ml_dtypes.bfloat16
    x = np.asarray(x, dtype=np.float32)
    fc_w = np.asarray(fc_w, dtype=np.float32)
    fc_b = np.asarray(fc_b, dtype=np.float32)
    avg_w = np.asarray(avg_w, dtype=np.float32)
    max_w = np.asarray(max_w, dtype=np.float32)
    gcn_w = np.asarray(gcn_w, dtype=np.float32)
    w3 = np.concatenate(
        [np.ascontiguousarray(fc_w.T), np.ascontiguousarray(avg_w.T),
         np.ascontiguousarray(max_w.T)], axis=1,
    )
    shared = {
        "w3": np.ascontiguousarray(w3).astype(b16),
        "fcb": np.ascontiguousarray(fc_b.reshape(M, 1)),
        "gcn": np.ascontiguousarray(gcn_w).astype(b16),
        "ident": np.eye(128, dtype=np.float32).astype(b16),
    }
    return [
        {"xT": np.ascontiguousarray(x[b].T).astype(b16), **shared}
        for b in range(x.shape[0])
    ]


def kernel(x, fc_w, fc_b, avg_w, max_w, gcn_w):
    from concourse.bass_utils import run_bass_kernel_spmd

    nc = _get_nc()
    in_maps = make_in_maps(x, fc_w, fc_b, avg_w, max_w, gcn_w)
    res = run_bass_kernel_spmd(nc, in_maps, list(range(len(in_maps))))
    out = np.stack([np.asarray(res.results[b]["out"]) for b in range(len(in_maps))])
    return out.astype(np.float32)
